# revision 1
# baseline (speedup 1.0000x reference)
"""Trainium2 Bass kernel for the 2-layer LSTM (H=51 -> H=1) over T=2048 steps.

Data-parallel over batch: 8 cores x 128 batch (batch on the free dim).
Per core/step: all gate pre-activations for BOTH layers land in one PSUM tile
P (128,256): I-block at partitions 0:52 / F at 64:116 (cols 0:128), O / G
(cols 128:256). tanh(z)=2*sigmoid(2z)-1 with the x2 folded into g weights, so
ONE Sigmoid covers all gates; one Tanh covers both cell rows. Layer 2 lags one
step. x_t enters via rank-1 matmuls from a flat partition-0 X stripe; y rows
leave via direct SBUF->HBM DMA. State tile R alternates parity for slack.
"""

import numpy as np

H = 51
B = 128
NCORES = 8
N_FULL = 1024
T_FULL = 2048
XB = 64          # time steps per X stripe


def pack_weights(W_ih1, W_hh1, b_ih1, b_hh1, W_ih2, W_hh2, b_ih2, b_hh2):
    """lhsT packs. K rows: 0:51 h1, 51 h2, 52 const-1(bias). M cols: gate
    blocks at 0:52 and 64:116 (pad to partition-64 alignment). G blocks x2."""
    def block(l1_rows, l2_row, scale):
        L = np.zeros((53, 52), np.float32)
        L[0:51, 0:51] = W_hh1[l1_rows, :].T
        L[0:51, 51] = W_ih2[l2_row, :]
        L[51, 51] = W_hh2[l2_row, 0]
        L[52, 0:51] = b_ih1[l1_rows] + b_hh1[l1_rows]
        L[52, 51] = b_ih2[l2_row] + b_hh2[l2_row]
        wx = np.zeros((52,), np.float32)
        wx[0:51] = W_ih1[l1_rows, 0]
        return L * scale, wx * scale

    L_I, wx_I = block(slice(0, 51), 0, 1.0)
    L_F, wx_F = block(slice(51, 102), 1, 1.0)
    L_G, wx_G = block(slice(102, 153), 2, 2.0)
    L_O, wx_O = block(slice(153, 204), 3, 1.0)

    A = np.concatenate([L_I, L_F, L_O, L_G], axis=1)          # (53, 208)
    LX = np.concatenate([wx_I, wx_F, wx_O, wx_G]).reshape(1, 208)
    return {"A_ALL": A, "LX_ALL": LX}


def build_program(T=T_FULL, debug=False):
    import concourse.bass as bass
    import concourse.tile as tile
    from concourse import bacc, mybir

    dt = mybir.dt.float32
    nc = bacc.Bacc("TRN2", target_bir_lowering=False, debug=debug)

    nxb = T // XB
    xT_d = nc.dram_tensor("xT", [nxb, XB * B], dt, kind="ExternalInput")
    yT_d = nc.dram_tensor("yT", [T, B], dt, kind="ExternalOutput")
    A_ALL_d = nc.dram_tensor("A_ALL", [53, 208], dt, kind="ExternalInput")
    LX_ALL_d = nc.dram_tensor("LX_ALL", [1, 208], dt, kind="ExternalInput")

    SIG = mybir.ActivationFunctionType.Sigmoid
    TANH = mybir.ActivationFunctionType.Tanh
    MUL = mybir.AluOpType.mult
    SUB = mybir.AluOpType.subtract

    with tile.TileContext(nc) as tc:
        with (
            tc.tile_pool(name="wts", bufs=1) as wpool,
            tc.tile_pool(name="state", bufs=1) as stpool,
            tc.tile_pool(name="xin", bufs=3) as xpool,
            tc.tile_pool(name="sg", bufs=2) as spool,
            tc.tile_pool(name="tmp", bufs=2) as tpool,
            tc.tile_pool(name="ps", bufs=2, space=bass.MemorySpace.PSUM) as ppool,
        ):
            A_ALL = wpool.tile([53, 208], dt, tag="aall")
            LX_ALL = wpool.tile([1, 208], dt, tag="lxall")
            nc.sync.dma_start(A_ALL[:], A_ALL_d[:])
            nc.sync.dma_start(LX_ALL[:], LX_ALL_d[:])

            ones = wpool.tile([1, B], dt, tag="ones")
            zrow = wpool.tile([1, B], dt, tag="zrow")
            nc.vector.memset(ones[:], 1.0)
            nc.vector.memset(zrow[:], 0.0)

            # state: R parity pair (53,B): 0:51 h1, 51 h2, 52 const-1
            R0 = stpool.tile([53, B], dt, tag="R0")
            R1 = stpool.tile([53, B], dt, tag="R1")
            Rp = [R0, R1]
            cc = stpool.tile([52, B], dt, tag="cc")
            nc.vector.memset(Rp[0][:], 0.0)
            nc.vector.memset(cc[:], 0.0)
            nc.sync.dma_start(Rp[0][52:53, :], ones[:])
            nc.sync.dma_start(Rp[1][52:53, :], ones[:])

            cur_x = None
            n_steps = T + 1  # device steps 0..T; layer 2 lags by one

            for s in range(n_steps):
                if s % XB == 0 and s < T:
                    cur_x = xpool.tile([1, XB * B], dt, tag="X")
                    nc.sync.dma_start(cur_x[:], xT_d[s // XB:s // XB + 1, :])

                Rin = Rp[s % 2]
                Rout = Rp[(s + 1) % 2]

                # y row: R_in[51] = h2(s-2), written by v5(s-1), safe 2 steps
                if s >= 2:
                    nc.sync.dma_start(yT_d[s - 2:s - 1, :], Rin[51:52, :])

                P = ppool.tile([52, 4 * B], dt, tag="P")
                for g in range(4):
                    Pg = P[:, g * B:(g + 1) * B]
                    Ag = A_ALL[:, g * 52:(g + 1) * 52]
                    if s < T:
                        xr = cur_x[0:1, (s % XB) * B:(s % XB + 1) * B]
                        nc.tensor.matmul(Pg, LX_ALL[0:1, g * 52:(g + 1) * 52],
                                         xr, start=True, stop=False)
                        nc.tensor.matmul(Pg, Ag, Rin[:], start=False, stop=True)
                    else:
                        nc.tensor.matmul(Pg, Ag, Rin[:], start=True, stop=True)

                S = spool.tile([52, 4 * B], dt, tag="S")
                nc.scalar.activation(S[:], P[:], SIG)
                s_I = S[:, 0:B]
                s_F = S[:, B:2 * B]
                s_O = S[:, 2 * B:3 * B]
                s_G = S[:, 3 * B:4 * B]

                m = tpool.tile([52, B], dt, tag="m")
                t1 = tpool.tile([52, B], dt, tag="t1")
                t2 = tpool.tile([52, B], dt, tag="t2")
                tau = tpool.tile([52, B], dt, tag="tau")
                nc.vector.tensor_mul(m[:], s_I, s_G)
                nc.vector.scalar_tensor_tensor(t1[:], m[:], 2.0, s_I,
                                               op0=MUL, op1=SUB)
                nc.vector.tensor_mul(t2[:], s_F, cc[:])
                nc.vector.tensor_add(cc[:], t1[:], t2[:])
                if s == 0:
                    nc.sync.dma_start(cc[51:52, :], zrow[:])  # c2 lag fix
                nc.scalar.activation(tau[:], cc[:], TANH)
                nc.vector.tensor_mul(Rout[0:52, :], s_O, tau[:])
                if s == 0:
                    nc.sync.dma_start(Rout[51:52, :], zrow[:])  # h2 lag fix

            # final row: y[T-1] = h2(T-1), in R[(T+1)%2][51] after step T
            nc.sync.dma_start(yT_d[T - 1:T, :], Rp[(T + 1) % 2][51:52, :])

    nc.compile()
    return nc


def kernel(stimulus, W_ih1, W_hh1, b_ih1, b_hh1, W_ih2, W_hh2, b_ih2, b_hh2):
    from concourse.bass_utils import run_bass_kernel_spmd

    N, T = stimulus.shape
    assert (N, T) == (N_FULL, T_FULL)
    pk = pack_weights(W_ih1, W_hh1, b_ih1, b_hh1, W_ih2, W_hh2, b_ih2, b_hh2)
    xT = np.ascontiguousarray(stimulus.T.astype(np.float32))  # (T, N)

    nc = build_program(T=T)
    in_maps = []
    for c in range(NCORES):
        xc = np.ascontiguousarray(xT[:, c * B:(c + 1) * B])
        m = {"xT": xc.reshape(T // XB, XB * B)}
        m.update(pk)
        in_maps.append(m)
    res = run_bass_kernel_spmd(nc, in_maps, list(range(NCORES)))
    yT = np.concatenate([res.results[c]["yT"] for c in range(NCORES)], axis=1)
    return np.ascontiguousarray(yT.T)  # (N, T)

